# revision 9
# baseline (speedup 1.0000x reference)
"""Capsule-routing kernel for Trainium2 (8 NeuronCores, data-parallel over batch).

Reference (per item, S=512 input caps, N=32 output caps, D=64, 3 iters):
    u_hat = (u @ W).reshape(S, N, D)        # never materialized
    b = 0
    for it in 0..2:
        c = softmax(b, axis=caps)
        o = squash(einsum('ns,nsd->nd', c, u_hat))   # squash = L2 normalize
        if it < 2: b = einsum('nd,nsd->ns', o, u_hat)

Re-association (per item):
    mT[i, n] = sum_s u[s,i] c[n,s]            (m-step, contract s)
    o[n, d]  = sum_i mT[i,n] W[i, n*64+d]     (o-step, block-diag, contract i)
    P[i, n]  = sum_d W[i, n*64+d] o[n,d]      (P-step, contract d via W^T)
    r[n, s]  = sum_i P[i,n] u[s,i]            (b-step, contract i via u^T)

Everything runs in bf16 on the PE (fp32 PSUM accumulate).

Schedule (cost-model informed):
  - All input DMAs are SWDGE cast-loads (fp32->bf16): DMA time is out-bytes /
    360 GB/s, so casting halves it. W loads in 2 halves (W^T build starts at
    ~6us instead of ~9.4us); u loads in 4 two-item chunks.
  - NO DMA transposes: they serialize with the loads on the shared DMA
    engines (the old kernel lost ~7us there). All W^T/u^T via PE transposes
    (53-107ns each); PSUM->SBUF copies are spread across DVE/Act/Pool.
  - Routing pipelined in 2 groups of 4 items; per-stage engines chosen to
    keep Act (the busiest vector engine) off the fat copies:
    mt copy -> DVE, oTu -> Pool, squash norm -> Act, scale -> DVE,
    pt -> Act, softmax exp -> Act, reduce/recip/mult -> DVE.
  - PE p-state warmers before the final iteration keep the 2.4GHz clock.
"""

import sys

import numpy as np

if "/opt/trn_rl_repo" not in sys.path:
    sys.path.insert(0, "/opt/trn_rl_repo")

import concourse.bass as bass  # noqa: F401
import concourse.mybir as mybir
import concourse.tile as tile
from concourse import bacc
from concourse.masks import make_identity

# Keep Exp/Ln/Square/Copy resolvable via one activation table so the kernel
# needs a single LoadActFuncSet (table swaps cost ~1.3us each).
_orig_get_tables = bacc.get_activation_tables


def _tables_prefer_nle(arch):
    t = _orig_get_tables(arch)
    pref = "natural_log_exp_and_others"
    if pref not in t:
        return t
    mine = t[pref]
    return {k: (v if k == pref else v - mine) for k, v in t.items()}


bacc.get_activation_tables = _tables_prefer_nle

FP = mybir.dt.float32
BF = mybir.dt.bfloat16
EPS = 1e-7
B, S, I = 64, 512, 512          # full batch, input caps, input dim
N, D = 32, 64                   # output caps, cap dim
NCORES = 8
BC = B // NCORES                # items per core = 8
G = 2                           # routing groups
BG = BC // G                    # items per group = 4
P = 128
IC = I // P                     # i chunks = 4
J = 4                           # s = 4*p + j
ROUTINGS = 3
ND = N * D


def _ap(base, offset_delta, dims):
    return bass.AP(tensor=base.tensor, offset=base.offset + offset_delta,
                   ap=dims)


def _copy(eng, nc, dst, src):
    if eng is nc.scalar:
        eng.copy(dst, src)
    else:
        eng.tensor_copy(dst, src)


def build_kernel(nc):
    u_dram = nc.dram_tensor("u", [BC, S, I], FP, kind="ExternalInput").ap()
    w_dram = nc.dram_tensor("W", [I, ND], FP, kind="ExternalInput").ap()
    o_dram = nc.dram_tensor("out", [BC, N, D], FP, kind="ExternalOutput").ap()

    with tile.TileContext(nc) as tc:
        _body(tc, u_dram, w_dram, o_dram)
    return nc


def _body(tc, u_dram, w_dram, o_dram):
    from contextlib import ExitStack

    nc = tc.nc
    ctx = ExitStack()
    with ctx:
        statics = ctx.enter_context(tc.tile_pool(name="statics", bufs=1))
        stage = ctx.enter_context(tc.tile_pool(name="stage", bufs=3))
        psum = ctx.enter_context(tc.tile_pool(name="psum", bufs=2, space="PSUM"))

        # ---------- statics ----------
        ident_f = statics.tile([P, P], FP)
        make_identity(nc, ident_f)
        ident = statics.tile([P, P], BF)
        nc.vector.tensor_copy(ident, ident_f)
        eps_sb = statics.tile([P, 1], FP)
        nc.vector.memset(eps_sb, EPS)

        w_bf = statics.tile([P, IC, ND], BF)     # W[128*ic+p, nd]
        wt = statics.tile([P, N // 2, I], BF)    # W[i, 128*q+p] at [p, q, i]
        u_bf = statics.tile([P, BC, J, I], BF)   # u[b, 4p+j, i]
        ut = statics.tile([P, BC, J * IC, P], BF)  # u[b,4q+j,128*ic+v] at [v,b,4j+ic,q]
        ct = statics.tile([P, BC, J, N], BF)     # c[b, n, 4p+j]
        mt = statics.tile([P, IC, N, BC], BF)    # m[b, n, 128*ic+v] at [v, ic, n, b]
        bd = [statics.tile([P, N // 2, 2 * BG], BF, name=f"bd{g}")
              for g in range(G)]                 # o[4g+bi, 2q+h, d] at [64h+d, q, 4h+bi]

        ones_n = statics.tile([P, 1], BF)
        nc.vector.memset(ones_n, 1.0 / N)
        for g in range(G):
            nc.vector.memset(bd[g].rearrange("p a b -> p (a b)"), 0.0)

        # ---------- cast-loads (SWDGE converts fp32 -> bf16 in the DMA) ----------
        # 6 SWDGE DMAs total (8 completion sems exist). W in halves so the
        # W^T build starts ~3us earlier; u in 2-item chunks for pipelining.
        with tc.high_priority(offset=-2000):
            nc.gpsimd.dma_start(
                out=w_bf[:, 0:2],
                in_=w_dram[0:2 * P].rearrange("(c p) n -> p c n", p=P))
            nc.gpsimd.dma_start(
                out=w_bf[:, 2:4],
                in_=w_dram[2 * P:4 * P].rearrange("(c p) n -> p c n", p=P))
            for ch in range(BC // 2):
                nc.gpsimd.dma_start(
                    out=u_bf[:, 2 * ch:2 * ch + 2],
                    in_=u_dram[2 * ch:2 * ch + 2].rearrange(
                        "b (p j) i -> p b j i", j=J))

        ones64 = statics.tile([P, 1], BF)
        nc.vector.memset(ones64, 1.0)

        # ---------- W^T + u^T builds (PE transposes; copies spread) ----------
        # GPSIMD cannot read PSUM, so the PSUM->SBUF copies alternate between
        # DVE (fastest per col) and Act, DVE-weighted.
        wt_copy_engines = [nc.vector, nc.scalar, nc.vector, nc.vector,
                           nc.scalar, nc.vector, nc.vector, nc.scalar] * 2
        ut_copy_engines = [nc.vector, nc.scalar, nc.vector, nc.scalar,
                           nc.vector, nc.scalar, nc.vector, nc.vector,
                           nc.scalar, nc.vector, nc.scalar, nc.vector,
                           nc.scalar, nc.vector, nc.vector, nc.vector]

        with tc.high_priority(offset=-1000):
            # W^T per half h (i-chunks 2h, 2h+1): fills wt[:, :, 256h:256(h+1)]
            k = 0
            for h in range(2):
                for qq in range(N // 4):
                    tbw = psum.tile([P, 512], BF, tag="tp", name="tbw", bufs=3)
                    for dq in range(2):
                        q = 2 * qq + dq
                        for icl in range(2):
                            ic = 2 * h + icl
                            nc.tensor.transpose(
                                tbw[:, (dq * 2 + icl) * P:(dq * 2 + icl + 1) * P],
                                w_bf[:, ic, q * P:(q + 1) * P], ident)
                    dst = _ap(wt, (2 * qq) * I + 2 * h * P,
                              [wt.ap[0], [I, 2], [1, 2 * P]])
                    _copy(wt_copy_engines[k], nc, dst,
                          tbw.rearrange("p (a b) -> p a b", a=2))
                    k += 1
            # u^T per item (two halves of j each)
            k = 0
            for b in range(BC):
                for half in range(2):
                    tbu = psum.tile([P, 1024], BF, tag="tp", name="tbu", bufs=3)
                    for jj in range(2):
                        j = 2 * half + jj
                        for ic in range(IC):
                            nc.tensor.transpose(
                                tbu[:, (jj * IC + ic) * P:(jj * IC + ic + 1) * P],
                                u_bf[:, b, j, ic * P:(ic + 1) * P], ident)
                    _copy(ut_copy_engines[k], nc,
                          ut[:, b, half * 2 * IC:(half + 1) * 2 * IC, :], tbu)
                    k += 1

        # PE p-state warmers before the final iteration: junk transposes in
        # the natural PE idle window so it2's matmuls run at full clock.
        junk = psum.tile([P, 1024], BF, tag="tp", name="junk", bufs=3)

        def fill(k):
            for _ in range(k):
                nc.tensor.transpose(junk[:, 0:P], ident, ident)

        # ---------- routing ----------
        for it in range(ROUTINGS):
            for g in range(G):
                if it == ROUTINGS - 1:
                    fill(20 if g == 0 else 26)
                _route_iter(tc, stage, psum, o_dram, it, g, ident, eps_sb,
                            w_bf, wt, u_bf, ut, ct, mt, bd, ones_n, ones64)


def _route_iter(tc, stage, psum, o_dram, it, g, ident, eps_sb,
                w_bf, wt, u_bf, ut, ct, mt, bd, ones_n, ones64):
    nc = tc.nc
    last = it == ROUTINGS - 1

    # m-step: mT[v, ic, n] per item; contract s = (p, j) on partitions.
    # One PSUM bank holds the whole group's m.
    pm = psum.tile([P, BG, IC, N], FP, tag="pm", name="pm", bufs=1)
    if it == 0:
        # c == 1/N exactly at iter 0: m0[i] = (1/N) sum_s u[s,i] is the same
        # for every cap -> one column per (item, i-chunk) via a ones-vector.
        for bi in range(BG):
            b = BG * g + bi
            for ic in range(IC):
                for j in range(J):
                    nc.tensor.matmul(
                        pm[:, bi, ic, 0:1],
                        lhsT=u_bf[:, b, j, ic * P:(ic + 1) * P],
                        rhs=ones_n,
                        start=(j == 0), stop=(j == J - 1))
        nc.vector.tensor_copy(
            mt[:, :, 0, BG * g:BG * (g + 1)],
            pm[:, :, :, 0].rearrange("p b i -> p i b"))
    else:
        for bi in range(BG):
            b = BG * g + bi
            for ic in range(IC):
                for j in range(J):
                    nc.tensor.matmul(
                        pm[:, bi, ic, :],
                        lhsT=u_bf[:, b, j, ic * P:(ic + 1) * P],
                        rhs=ct[:, b, j, :],
                        start=(j == 0), stop=(j == J - 1))
        nc.vector.tensor_copy(mt[:, :, :, BG * g:BG * (g + 1)],
                              pm.rearrange("p b i n -> p i n b"))

    # squash bank: ot (fp32, o-step dst) | on (bf16) | ots (bf16), carved
    # from one 2KB PSUM bank.
    sqb = psum.tile([P, 1024], BF, tag="sq", name="sqb", bufs=2)
    ot = sqb.bitcast(FP)[0:D, 0:N * BG]
    on_ps = sqb[:, 256:256 + D]
    oTs = sqb[0:D, 384:384 + P]

    # o-step: ot[d, 4n+bi] = sum_i mT[i,n] W[i, n*64+d] for the group
    for n in range(N):
        for ic in range(IC):
            n_src = 0 if it == 0 else n
            nc.tensor.matmul(
                ot[:, n * BG:(n + 1) * BG],
                lhsT=w_bf[:, ic, n * D:(n + 1) * D],
                rhs=mt[:, ic, n_src, BG * g:BG * (g + 1)],
                start=(ic == 0), stop=(ic == IC - 1))

    # squash: copy o^T to SBUF (Act), then in parallel
    #   - PE transposes it to (n,b)-on-partitions
    #   - Pool squares it (SBUF->SBUF) and PE row-reduces the squares -> n2
    oTu = stage.tile([D, N * BG], BF, tag="oTu", name="oTu")
    nc.scalar.copy(oTu, ot)
    nc.tensor.transpose(on_ps, oTu, ident[:D, :D])
    sq = stage.tile([D, N * BG], BF, tag="sq", name="sq")
    nc.gpsimd.tensor_tensor(sq, oTu, oTu, mybir.AluOpType.mult)
    n2p = sqb.bitcast(FP)[:, 320:321]
    nc.tensor.matmul(n2p, lhsT=sq, rhs=ones64[0:D], start=True, stop=True)
    lg = stage.tile([P, 1], FP, tag="lg", name="lg")
    nc.scalar.activation(lg, n2p, mybir.ActivationFunctionType.Ln,
                         bias=eps_sb[:, 0:1])
    rs = stage.tile([P, 1], FP, tag="rs", name="rs")
    nc.scalar.activation(rs, lg, mybir.ActivationFunctionType.Exp, scale=-0.5)
    rs_b = bass.AP(tensor=rs.tensor, offset=rs.offset, ap=[rs.ap[0], [0, D]])

    if last:
        onf = stage.tile([P, D], FP, tag="onf", name="onf")
        nc.scalar.activation(onf, on_ps, mybir.ActivationFunctionType.Copy,
                             scale=rs[:, 0:1])
        dst = _ap(o_dram, BG * g * N * D, [[D, N], [N * D, BG], [1, D]])
        nc.sync.dma_start(out=dst, in_=onf)
        return

    # scale on DVE (rs broadcast along free dim)
    onb = stage.tile([P, D], BF, tag="onb", name="onb")
    nc.vector.tensor_tensor(onb, on_ps, rs_b, mybir.AluOpType.mult)

    # block-diag o^T for the P-step: bd[64h+d, q, 4h+bi] = o[4g+bi, 2q+h, d]
    nc.tensor.transpose(oTs, onb, ident)
    oTs_v = oTs.rearrange("p (q x) -> p q x", q=N // 2)
    nc.vector.tensor_copy(bd[g][0:D, :, 0:BG], oTs_v[:, :, 0:BG])
    nc.vector.tensor_copy(bd[g][D:P, :, BG:2 * BG], oTs_v[:, :, BG:2 * BG])

    # P-step: P[i, (h,bi)] per pair q; contract (h,d) on partitions
    pp = psum.tile([P, IC, N // 2, 2 * BG], FP, tag="pp", name="pp", bufs=1)
    for ic in range(IC):
        for q in range(N // 2):
            nc.tensor.matmul(
                pp[:, ic, q, :],
                lhsT=wt[:, q, ic * P:(ic + 1) * P],
                rhs=bd[g][:, q, :],
                start=True, stop=True)
    pt = stage.tile([P, IC, N // 2, 2 * BG], BF, tag="pt", name="pt")
    nc.scalar.copy(pt, pp)

    # b-step, then one group-wide softmax: exp on Act, row-sums on DVE,
    # reciprocal + the normalize multiplies on Pool (all-SBUF operands).
    # The multiplies go per item pair so the next iteration's m-step can
    # start on the first pair while the second is still normalizing.
    rt = psum.tile([P, BG, J, N], FP, tag="rt", name="rt", bufs=1)
    for bi in range(BG):
        b = BG * g + bi
        for j in range(J):
            for ic in range(IC):
                rhs = _ap(pt, ic * (N // 2) * 2 * BG + bi,
                          [pt.ap[0], [2 * BG, N // 2], [BG, 2]])
                nc.tensor.matmul(
                    rt[:, bi, j, :],
                    lhsT=ut[:, b, j * IC + ic, :],
                    rhs=rhs,
                    start=(ic == 0), stop=(ic == IC - 1))
    et = stage.tile([P, BG * J, N], FP, tag="et", name="et", bufs=2)
    nc.scalar.activation(et, rt.rearrange("p b j n -> p (b j) n"),
                         mybir.ActivationFunctionType.Exp)
    zz = stage.tile([P, BG * J], FP, tag="zz", name="zz", bufs=2)
    nc.vector.reduce_sum(zz, et, axis=mybir.AxisListType.X)
    rz = stage.tile([P, BG * J], FP, tag="rz", name="rz", bufs=2)
    nc.vector.reciprocal(rz, zz)
    for half in range(2):
        b0 = BG * g + 2 * half
        rz_b = _ap(rz, 2 * half * J, [rz.ap[0], [1, 2 * J], [0, N]])
        nc.gpsimd.tensor_tensor(
            ct[:, b0:b0 + 2].rearrange("p b j n -> p (b j) n"),
            et[:, 2 * half * J:(half + 1) * 2 * J, :],
            rz_b, mybir.AluOpType.mult)


_COMPILED = None


def _get_compiled():
    global _COMPILED
    if _COMPILED is None:
        nc = bacc.Bacc("TRN2", target_bir_lowering=False, debug=False,
                       num_devices=NCORES,
                       dynamic_dma_scratch_size=49152)
        build_kernel(nc)
        nc.compile()
        _COMPILED = nc
    return _COMPILED


def kernel(u_vecs, W):
    from concourse.bass_utils import run_bass_kernel_spmd

    u_vecs = np.ascontiguousarray(u_vecs, dtype=np.float32)
    W = np.ascontiguousarray(W, dtype=np.float32)
    assert u_vecs.shape == (B, S, I) and W.shape == (I, ND)

    nc = _get_compiled()
    in_maps = [
        {"u": u_vecs[c * BC:(c + 1) * BC], "W": W} for c in range(NCORES)
    ]
    res = run_bass_kernel_spmd(nc, in_maps, list(range(NCORES)))
    return np.concatenate(
        [res.results[c]["out"] for c in range(NCORES)], axis=0
    ).astype(np.float32)
